# revision 1
# baseline (speedup 1.0000x reference)
"""AttnBlock (GroupNorm -> single-head attention -> proj -> residual) on 8
Trainium2 NeuronCores.

Sharding: core = (b, s); b = core // 4 selects the batch element, s = core % 4
selects a 2048-wide query slice of N=8192. Each core receives x[b] rolled by
-2048*s along N so its queries are always columns 0..2047 (keys become a
permutation of N, which softmax/attention are invariant to). This keeps one
SPMD program with static addressing and no collectives.

Layouts on-chip (partition dim first):
  x/h/k/q: [p=128, ci=2, n]  with channel c = 128*ci + p   (channels on partition)
  v:       [p=128, j=64, 257] with row n = 128*j + p, col 256 = 1.0 (denominator)
  attention: scoresT[j, m] tiles -> exp -> e (bf16); h2[m, c|denom] accumulated
  in PSUM over all 64 key tiles; normalized by the ones-column sum. Softmax max
  subtraction is skipped: scores*C^-0.5 is ~N(0,1), exp stays far from fp32
  range limits. bv is folded into the projection bias on the host
  (softmax rows sum to 1), so v needs no on-device bias.
"""

import ml_dtypes
import numpy as np

import bass_rust
import concourse.bass as bass
import concourse.tile as tile
from concourse import mybir
from concourse.bass_utils import run_bass_kernel_spmd

B, C, N = 2, 256, 8192
NCORES = 8
NSLICE = 4          # query slices per batch element
MQ = N // NSLICE    # 2048 queries per core
CHUNK = 512         # queries processed per attention pass
JT = N // 128       # 64 key tiles
EPS = 1e-5
SCALE = C ** -0.5   # 0.0625

F32 = mybir.dt.float32
BF16 = mybir.dt.bfloat16
FP8 = mybir.dt.float8e4
BF16_NP = ml_dtypes.bfloat16
FP8_NP = ml_dtypes.float8_e4m3
AX = mybir.AxisListType.X
AF = mybir.ActivationFunctionType


# ---------------------------------------------------------------------------
# Workaround: this container's walrus build rejects any instruction carrying
# more than one semaphore wait ("Too many sync wait commands"). Two pieces:
# (1) the Tile exit drain gets its waits split across per-proc sync nops;
# (2) a post-pass hoists excess waits from scheduled instructions onto
#     same-engine NoOps inserted immediately before them (same engine +
#     program order => identical blocking semantics).
def _drain_and_barrier_split(self, tick_clock, wait_clock):
    gc = tick_clock.global_clock
    vals = list(gc)
    n = len(vals)
    for i, v in enumerate(vals):
        if v == 0:
            continue
        vec = [0] * n
        vec[i] = v
        nop = self.nc.sync.nop(nofuse=True, hint=f"drain_split_{i}")
        wait_clock.add_sem_waits(
            nop.ins, bass_rust.ScopedClock({None: bass_rust.VectorClock(vec)})
        )
    self.nc.sync.drain()
    self.nc.all_engine_barrier()
    assert self.sems is not None
    popped = self.nc._tile_sem_poison_stack.pop()
    assert popped is self._sem_poison
    self.nc.clear_and_free_semaphores(list(self.sems.allocated().values()))
    self.nc.all_engine_barrier()


tile.TileContext._drain_and_barrier = _drain_and_barrier_split


def _split_excess_waits(nc, max_waits=1):
    for f in nc.m.functions:
        for blk in f.blocks:
            il = blk.instructions
            out = []
            changed = False
            for inst in il:
                si = getattr(inst, "sync_info", None)
                waits = list(si.on_wait) if si is not None and si.on_wait else []
                if len(waits) > max_waits:
                    for k, w in enumerate(waits[:-max_waits]):
                        nop = bass_rust.InstNoOp(
                            name=f"{inst.name}-wsplit{k}", ins=[], outs=[])
                        nop.engine = inst.engine
                        nop.sync_info = bass_rust.SyncInfo(
                            on_wait=[w], on_update=[])
                        out.append(nop)
                    si.on_wait = waits[-max_waits:]
                    changed = True
                out.append(inst)
            if changed:
                il[:] = out
# ---------------------------------------------------------------------------


def build_program() -> bass.Bass:
    nc = bass.Bass("TRN2", target_bir_lowering=False, debug=False)

    x_d = nc.dram_tensor("x", [128, 2, N], FP8, kind="ExternalInput").ap()
    xr_d = nc.dram_tensor("xres", [128, 2, MQ], F32, kind="ExternalInput").ap()
    wq_d = nc.dram_tensor("wq", [128, 2, 2, 128], FP8, kind="ExternalInput").ap()
    wk_d = nc.dram_tensor("wk", [128, 2, 2, 128], FP8, kind="ExternalInput").ap()
    wv_d = nc.dram_tensor("wv", [128, 2, 256], FP8, kind="ExternalInput").ap()
    wp_d = nc.dram_tensor("wp", [128, 2, 2, 128], BF16, kind="ExternalInput").ap()
    bq_d = nc.dram_tensor("bq", [128, 2], F32, kind="ExternalInput").ap()
    bk_d = nc.dram_tensor("bk", [128, 2], F32, kind="ExternalInput").ap()
    bp_d = nc.dram_tensor("bp", [128, 2], F32, kind="ExternalInput").ap()
    out_d = nc.dram_tensor("out", [128, 2, MQ], F32, kind="ExternalOutput").ap()

    with tile.TileContext(nc) as tc:
        with (
            tc.tile_pool(name="consts", bufs=1) as consts,
            tc.tile_pool(name="hsb", bufs=1) as hpool,
            tc.tile_pool(name="stats", bufs=1) as stats,
            tc.tile_pool(name="pp", bufs=2, space="PSUM") as pp,
            tc.tile_pool(name="ph2p", bufs=4, space="PSUM") as ph2p,
        ):
            # ---- constants -------------------------------------------------
            wq_sb = consts.tile([128, 2, 2, 128], FP8)
            wk_sb = consts.tile([128, 2, 2, 128], FP8)
            wv_sb = consts.tile([128, 2, 256], FP8)
            wp_sb = consts.tile([128, 2, 2, 128], BF16)
            bq_sb = consts.tile([128, 2], F32)
            bk_sb = consts.tile([128, 2], F32)
            bp_sb = consts.tile([128, 2], F32)
            ones_sb = consts.tile([128, 2, 16], FP8)
            onesf_sb = consts.tile([1, 128], F32)
            nb2_sb = consts.tile([128, 1], F32)
            for dst, srcap in [
                (wq_sb, wq_d), (wk_sb, wk_d), (wv_sb, wv_d), (wp_sb, wp_d),
                (bq_sb, bq_d), (bk_sb, bk_d), (bp_sb, bp_d),
            ]:
                nc.sync.dma_start(out=dst, in_=srcap)
            nc.vector.memset(ones_sb, 1.0)
            nc.vector.memset(onesf_sb, 1.0)
            nc.vector.memset(nb2_sb, -2.0)

            xt = hpool.tile([128, 2, N], FP8)
            for ci in range(2):
                for qd in range(4):
                    qsl = slice(qd * (N // 4), (qd + 1) * (N // 4))
                    nc.sync.dma_start(out=xt[:, ci, qsl], in_=x_d[:, ci, qsl])

            with (
                tc.tile_pool(name="kqv", bufs=1) as kqv,
                tc.tile_pool(name="esb", bufs=4) as epool,
                tc.tile_pool(name="tsb", bufs=2) as tpool,
                tc.tile_pool(name="osb", bufs=3) as opool,
            ):
                # ---- phase B: k, q (channels-on-partition) and v (rows) ----
                # PSUM drains alternate between ACT and DVE to keep both
                # engines below the PE's issue rate.
                kt = kqv.tile([128, 2, N], FP8)
                qt = kqv.tile([128, 2, MQ], FP8)
                vt = kqv.tile([128, JT // 2, 2, 256], FP8)
                DR = mybir.MatmulPerfMode.DoubleRow

                def drain_bias(idx, dst, ps, bias_ap):
                    # alternate engines so neither ACT nor DVE gates PE
                    if idx % 2 == 0:
                        nc.scalar.activation(out=dst, in_=ps, func=AF.Identity,
                                             bias=bias_ap)
                    else:
                        nc.vector.tensor_scalar_add(out=dst, in0=ps,
                                                    scalar1=bias_ap)

                for ot in range(2):
                    for np2 in range(N // 1024):
                        ps2 = pp.tile([128, 2, 512], F32, tag="ps",
                                      name=f"psk_{ot}_{np2}")
                        for r in range(2):
                            sl = slice(np2 * 1024 + r * 512,
                                       np2 * 1024 + r * 512 + 512)
                            nc.tensor.matmul(ps2[:, r, :],
                                             lhsT=wk_sb[:, :, ot, :],
                                             rhs=xt[:, :, sl], perf_mode=DR,
                                             start=True, stop=True)
                        osl = slice(np2 * 1024, np2 * 1024 + 1024)
                        drain_bias(np2, kt[:, ot, osl], ps2,
                                   bk_sb[:, ot:ot + 1])
                for ot in range(2):
                    for np2 in range(MQ // 1024):
                        ps2 = pp.tile([128, 2, 512], F32, tag="ps",
                                      name=f"psq_{ot}_{np2}")
                        for r in range(2):
                            sl = slice(np2 * 1024 + r * 512,
                                       np2 * 1024 + r * 512 + 512)
                            nc.tensor.matmul(ps2[:, r, :],
                                             lhsT=wq_sb[:, :, ot, :],
                                             rhs=xt[:, :, sl], perf_mode=DR,
                                             start=True, stop=True)
                        osl = slice(np2 * 1024, np2 * 1024 + 1024)
                        drain_bias(np2, qt[:, ot, osl], ps2,
                                   bq_sb[:, ot:ot + 1])
                for jj in range(JT // 2):
                    ps2 = pp.tile([128, 2, 512], F32, tag="ps",
                                  name=f"psv_{jj}")
                    for r in range(2):
                        j = 2 * jj + r
                        sl = slice(j * 128, j * 128 + 128)
                        nc.tensor.matmul(ps2[:, r, 0:256], lhsT=xt[:, :, sl],
                                         rhs=wv_sb, perf_mode=DR,
                                         start=True, stop=True)
                    if jj % 2 == 0:
                        nc.scalar.activation(out=vt[:, jj, :, :],
                                             in_=ps2[:, :, 0:256],
                                             func=AF.Copy)
                    else:
                        nc.vector.tensor_copy(out=vt[:, jj, :, :],
                                              in_=ps2[:, :, 0:256])

                # ---- phase C: attention + projection per 512-query chunk ---
                # h2 is accumulated directly in [c, m] layout (v as the
                # stationary operand); the softmax denominator comes from a
                # ones-row matmul and is applied to the projected output as a
                # broadcast reciprocal row.
                for mc in range(MQ // CHUNK):
                    msl = slice(mc * CHUNK, mc * CHUNK + CHUNK)
                    hc = [ph2p.tile([128, CHUNK], F32, tag="hcm", bufs=2,
                                    name=f"hc_{mc}_{i}") for i in range(2)]
                    den = ph2p.tile([1, CHUNK], F32, tag="den", bufs=1,
                                    name=f"den_{mc}")
                    for jj in range(JT // 2):
                        et = epool.tile([128, 2, CHUNK], FP8)
                        ps2 = pp.tile([128, 2, CHUNK], F32, tag="ps")
                        for r in range(2):
                            j = 2 * jj + r
                            jsl = slice(j * 128, j * 128 + 128)
                            nc.tensor.matmul(ps2[:, r, :], lhsT=kt[:, :, jsl],
                                             rhs=qt[:, :, msl], perf_mode=DR,
                                             start=True, stop=True)
                        # -2 bias keeps exp() well inside fp8e4m3 range; it
                        # cancels in the softmax normalizer.
                        nc.scalar.activation(out=et, in_=ps2, func=AF.Exp,
                                             scale=SCALE, bias=nb2_sb)
                        first, last = jj == 0, jj == JT // 2 - 1
                        for ci in range(2):
                            nc.tensor.matmul(
                                hc[ci],
                                lhsT=vt[:, jj, :, ci * 128:ci * 128 + 128],
                                rhs=et, perf_mode=DR,
                                start=first, stop=last)
                        nc.tensor.matmul(den, lhsT=ones_sb[:, :, 0:1],
                                         rhs=et, perf_mode=DR,
                                         start=first, stop=last)
                    # drain h2, build the reciprocal-denominator row
                    h2sb = tpool.tile([128, 2, CHUNK], BF16)
                    for ci in range(2):
                        nc.vector.tensor_copy(out=h2sb[:, ci, :], in_=hc[ci])
                    rd_row = stats.tile([1, CHUNK], F32, tag="rdrow")
                    nc.vector.reciprocal(rd_row, den)
                    # broadcast the reciprocal row to all 128 partitions with
                    # a K=1 ones matmul, then stage it in SBUF
                    ps_bc = pp.tile([128, CHUNK], F32, tag="ps",
                                    name=f"psbc_{mc}")
                    nc.tensor.matmul(ps_bc, lhsT=onesf_sb, rhs=rd_row,
                                     start=True, stop=True)
                    rdb = opool.tile([128, CHUNK], F32, tag="rdb")
                    nc.vector.tensor_copy(out=rdb, in_=ps_bc)
                    # projection + denominator + bias + residual
                    for ot in range(2):
                        ps_o = pp.tile([128, CHUNK], F32, tag="ps")
                        for ci in range(2):
                            nc.tensor.matmul(ps_o, lhsT=wp_sb[:, ci, ot, :],
                                             rhs=h2sb[:, ci, :],
                                             start=(ci == 0), stop=(ci == 1))
                        o_sb = opool.tile([128, CHUNK], F32, tag="o_sb")
                        nc.vector.tensor_mul(o_sb, ps_o, rdb)
                        xr = opool.tile([128, CHUNK], F32, tag="xr")
                        nc.sync.dma_start(out=xr, in_=xr_d[:, ot, msl])
                        nc.vector.scalar_tensor_tensor(
                            out=o_sb, in0=o_sb, scalar=bp_sb[:, ot:ot + 1],
                            in1=xr, op0=mybir.AluOpType.add,
                            op1=mybir.AluOpType.add)
                        nc.sync.dma_start(out=out_d[:, ot, msl], in_=o_sb)
    _split_excess_waits(nc)
    return nc


_NC_CACHE = None


def _get_program():
    global _NC_CACHE
    if _NC_CACHE is None:
        _NC_CACHE = build_program()
    return _NC_CACHE


def _prep_batch(inputs, b, x):
    """Fold GroupNorm (stats computed here on the host) into the q/k/v
    weights and biases for batch element b: h = s1*x + s2 per channel, so
    W @ h = (W*diag(s1)) @ x + W @ s2."""
    f32 = np.float32
    wq = np.asarray(inputs["wq"], f32)
    wk = np.asarray(inputs["wk"], f32)
    wv = np.asarray(inputs["wv"], f32)
    wp = np.asarray(inputs["wp"], f32)
    bv = np.asarray(inputs["bv"], f32)
    bp = np.asarray(inputs["bp"], f32)
    gw = np.asarray(inputs["gn_weight"], f32)
    gb = np.asarray(inputs["gn_bias"], f32)

    g = x[b].reshape(32, 8 * N)
    mean = g.mean(axis=1)
    var = g.var(axis=1)
    rstd = 1.0 / np.sqrt(var + EPS)
    s1 = np.repeat(rstd, 8) * gw                       # [C]
    s2 = gb - np.repeat(mean * rstd, 8) * gw           # [C]

    wq_f = wq * s1[None, :]
    wk_f = wk * s1[None, :]
    wv_f = wv * s1[None, :]
    bq_f = np.asarray(inputs["bq"], f32) + wq @ s2
    bk_f = np.asarray(inputs["bk"], f32) + wk @ s2
    # v's constant part rides through softmax (rows sum to 1) into the
    # projection bias: bp_eff = bp + wp @ (bv + wv @ s2)
    bp_f = bp + wp @ (bv + wv @ s2)

    def wT_pack(w, dt):  # [o, c] -> [p, ci, ot, o_local] of w.T
        return np.ascontiguousarray(
            w.T.reshape(2, 128, 2, 128).transpose(1, 0, 2, 3)
        ).astype(dt)

    return {
        "wq": wT_pack(wq_f, FP8_NP),
        "wk": wT_pack(wk_f, FP8_NP),
        "wp": wT_pack(wp, BF16_NP),
        "wv": np.ascontiguousarray(
            wv_f.T.reshape(2, 128, 256).transpose(1, 0, 2)
        ).astype(FP8_NP),
        "bq": np.ascontiguousarray(bq_f.reshape(2, 128).T),
        "bk": np.ascontiguousarray(bk_f.reshape(2, 128).T),
        "bp": np.ascontiguousarray(bp_f.reshape(2, 128).T),
    }


def kernel(**inputs) -> np.ndarray:
    x = np.asarray(inputs["x"], np.float32)  # [B, C, N]

    in_maps = []
    for b in range(B):
        shared_b = _prep_batch(inputs, b, x)
        xb8 = x[b].astype(FP8_NP)  # convert once, roll per slice
        for s in range(NSLICE):
            xr8 = np.roll(xb8, -MQ * s, axis=1)  # queries at columns 0..MQ-1
            x_in = np.ascontiguousarray(
                xr8.reshape(2, 128, N).transpose(1, 0, 2))
            xres = np.ascontiguousarray(
                x[b][:, MQ * s:MQ * (s + 1)]
                .reshape(2, 128, MQ).transpose(1, 0, 2))
            in_maps.append({"x": x_in, "xres": xres, **shared_b})

    nc = _get_program()
    res = run_bass_kernel_spmd(nc, in_maps, core_ids=list(range(NCORES)))

    out = np.empty((B, C, N), np.float32)
    for core in range(NCORES):
        b, s = divmod(core, NSLICE)
        r = res.results[core]["out"]  # [128, 2, MQ]
        out[b][:, MQ * s:MQ * (s + 1)] = r.transpose(1, 0, 2).reshape(C, MQ)
    return out

